# revision 22
# baseline (speedup 1.0000x reference)
"""Expert-parallel CMoE kernel for 8 Trainium2 NeuronCores.

Sharding (hardcoded for B=8, T=2048, D=1024, F=2048, E=16, C=1024):
  core k owns batch k (token shift, receptance, output) and experts
  {2k, 2k+1} (FFN). Hash routing is int math on token_ids, done on host;
  the resulting permutations ship to the cores as index tensors.

v6 design:
  - x ships transposed (d-major, bf16) with the shift state as column 0,
    so the token-shift mix is a free-axis slice: one DMA read of x.
  - xk/xr computed in d-major; receptance consumes xrT from SBUF and r
    stays resident in SBUF for phase D.
  - xk is PE-transposed back to token-major for the dispatch scatter.
  - consecutive indirect scatters to one DRAM tile serialize on DMA
    completion (~8us/hop), so dispatch uses FOUR buffers (half x tile
    parity) and combine TWO per expert (slot-tile parity): chains halve
    and the smaller AllToAlls pipeline on the CC core.
  - receptance matmuls hide the dispatch collectives (2 run after the
    first pair fires, 2 after the second pair).
  - phase C per expert: transposing dma_gather -> FFN1 -> relu^2 ->
    FFN2 -> indirect scatter (parity-split) -> two combine AllToAlls.
  - phase D runs twice in natural token order: pass 0 (expert-parity 0,
    other tokens hit the zeroed trash row) lands during the expert-1
    combine collectives; pass 1 DMA-accumulates into out after them.
All matmuls bf16 with fp32 PSUM accumulation; dropped tokens and empty
expert slots route through zeroed trash rows.
"""
import sys

for _p in ("/opt/trn_rl_repo", "/root/.axon_site/_ro/trn_rl_repo"):
    if _p not in sys.path:
        sys.path.append(_p)

import numpy as np
import ml_dtypes

import concourse.bass as bass
import concourse.bacc as bacc
import concourse.mybir as mybir
import concourse.tile as tile
from concourse import masks
from concourse.bass_utils import run_bass_kernel_spmd

P = 128
B, T, D, F, E = 8, 2048, 1024, 2048, 16
N = B * T
C = max(4, N // E)          # 1024
HASH_PRIME = 5099
NCORES = 8
EPC = E // NCORES           # experts per core = 2
DC = D // P                 # 8
FC = F // P                 # 16
BF16 = mybir.dt.bfloat16
F32 = mybir.dt.float32
I16 = mybir.dt.int16
I32 = mybir.dt.int32
nbf16 = ml_dtypes.bfloat16
AF = mybir.ActivationFunctionType
ALU = mybir.AluOpType

_CACHE = {}


def _r16(v):
    return int(-(-int(v) // 16) * 16)


def _wrap16(a):
    a = np.asarray(a, np.int16)
    w = a.reshape(-1, 16).T.copy()       # j at [j%16, j//16]
    return np.tile(w, (8, 1))            # replicated across 8 Q7 cores


def _route(token_ids):
    tid = np.asarray(token_ids).reshape(N).astype(np.int64)
    e = (tid * HASH_PRIME) % E
    onehot = (e[:, None] == np.arange(E)).astype(np.int64)
    pos = onehot.cumsum(0)[np.arange(N), e] - 1
    keep = pos < C
    return e, pos, keep


def _build_indices(token_ids):
    e, pos, keep = _route(token_ids)
    src = np.arange(N) // T
    local_t = np.arange(N) % T
    dst = e // EPC
    el = e % EPC
    tile_i = local_t // P

    def pack(mask):
        rank = np.zeros(N, np.int64)
        cnt = np.zeros((NCORES, NCORES), np.int64)
        for n in np.nonzero(mask)[0]:
            rank[n] = cnt[src[n], dst[n]]
            cnt[src[n], dst[n]] += 1
        return rank, _r16(cnt.max())

    # ---- dispatch: 4 groups = (half, tile parity); the first pair of
    # collectives fires mid-phase-A, chains per buffer are 4 hops
    dgrp = (local_t >= T // 2) * 2 + (tile_i % 2)
    dK = []
    drank = np.zeros(N, np.int64)
    for g in range(4):
        m = keep & (dgrp == g)
        r_, K_ = pack(m)
        dK.append(K_)
        drank[m] = r_[m]
    srcoff = np.zeros(N, np.int64)
    for g in range(4):
        m = dgrp == g
        srcoff[m & keep] = dst[m & keep] * dK[g] + drank[m & keep]
        srcoff[m & ~keep] = NCORES * dK[g]
    dbase = [0]
    for g in range(4):
        dbase.append(dbase[-1] + NCORES * dK[g])
    ZR1 = dbase[4]
    recv_row = np.full((NCORES, EPC * C), ZR1, np.int64)
    for n in np.nonzero(keep)[0]:
        recv_row[dst[n], el[n] * C + pos[n]] = \
            dbase[dgrp[n]] + src[n] * dK[dgrp[n]] + drank[n]

    # ---- combine: 2 groups per el = slot-tile parity; note sender is
    # the expert owner (dst), receiver the token owner (src)
    cpar = (pos // P) % 2
    cK = {}
    crank = np.zeros(N, np.int64)
    for eli in range(EPC):
        for par in range(2):
            m = keep & (el == eli) & (cpar == par)
            r_, K_ = pack(m)
            cK[(eli, par)] = K_
            crank[m] = r_[m]
    sl2 = np.zeros((NCORES, EPC, C), np.int64)
    for eli in range(EPC):
        for cpos in range(C):
            sl2[:, eli, cpos] = NCORES * cK[(eli, (cpos // P) % 2)]
    ygath = {eli: np.full(N, NCORES * (cK[(eli, 0)] + cK[(eli, 1)]),
                          np.int64) for eli in range(EPC)}
    for n in np.nonzero(keep)[0]:
        eli, par = el[n], cpar[n]
        sl2[dst[n], eli, pos[n]] = src[n] * cK[(eli, par)] + crank[n]
        ygath[eli][n] = par * NCORES * cK[(eli, 0)] + \
            dst[n] * cK[(eli, par)] + crank[n]

    per_core = []
    for k in range(NCORES):
        tok = slice(k * T, (k + 1) * T)
        per_core.append({
            "src32": srcoff[tok].astype(np.int32).reshape(T // P, P).T.copy(),
            "slot16": _wrap16(recv_row[k]),
            "sl2_32": np.concatenate(
                [sl2[k, 0].reshape(-1, P).T, sl2[k, 1].reshape(-1, P).T],
                axis=1).astype(np.int32).copy(),
            "yg16_0": _wrap16(ygath[0][tok]),
            "yg16_1": _wrap16(ygath[1][tok]),
        })
    return (tuple(dK), cK[(0, 0)], cK[(0, 1)], cK[(1, 0)], cK[(1, 1)]), \
        per_core


def _build_nc(cfg):
    dK, K0e, K0o, K1e, K1o = cfg
    dK = list(dK)
    dbase = [0]
    for g in range(4):
        dbase.append(dbase[-1] + NCORES * dK[g])
    ZR1 = dbase[4]
    cK = {(0, 0): K0e, (0, 1): K0o, (1, 0): K1e, (1, 1): K1o}
    R2 = {0: NCORES * (K0e + K0o), 1: NCORES * (K1e + K1o)}
    nc = bacc.Bacc("TRN2", target_bir_lowering=False, debug=False,
                   num_devices=NCORES)

    x_t = nc.dram_tensor("x_t", [P, DC, T + 1], BF16, kind="ExternalInput")
    maa2 = nc.dram_tensor("maa2", [P, 2 * DC], BF16, kind="ExternalInput")
    wrt = nc.dram_tensor("wrt", [D, D], BF16, kind="ExternalInput")
    wk = nc.dram_tensor("wk", [EPC, D, F], BF16, kind="ExternalInput")
    wv = nc.dram_tensor("wv", [EPC, F, D], BF16, kind="ExternalInput")
    src32 = nc.dram_tensor("src32", [P, T // P], I32, kind="ExternalInput")
    slot16 = nc.dram_tensor("slot16", [P, EPC * C // 16], I16,
                            kind="ExternalInput")
    sl2_32 = nc.dram_tensor("sl2_32", [P, EPC * C // P], I32,
                            kind="ExternalInput")
    yg16_0d = nc.dram_tensor("yg16_0", [P, T // 16], I16,
                             kind="ExternalInput")
    yg16_1d = nc.dram_tensor("yg16_1", [P, T // 16], I16,
                             kind="ExternalInput")
    out = nc.dram_tensor("out", [T, D], F32, kind="ExternalOutput")

    rg = [list(range(NCORES))]

    with tile.TileContext(nc) as tc:
        with (
            tc.tile_pool(name="dram", bufs=1, space="DRAM") as dram,
            tc.tile_pool(name="misc", bufs=1) as misc,
            tc.tile_pool(name="pwk", bufs=1) as pwk,
            tc.tile_pool(name="pwv", bufs=1) as pwv,
        ):
            a1 = {g: dram.tile([NCORES * dK[g] + 1, D], BF16,
                               name=f"a1_{g}") for g in range(4)}
            recv1 = dram.tile([ZR1 + 1, D], BF16)
            a2 = {(eli, par): dram.tile(
                [NCORES * cK[(eli, par)] + 1, D], BF16,
                name=f"a2_{eli}{par}") for eli in range(EPC)
                for par in range(2)}
            recv2 = {eli: dram.tile([R2[eli] + 1, D], BF16,
                                    name=f"recv2_{eli}")
                     for eli in range(EPC)}

            zrow = misc.tile([1, D], BF16)
            nc.vector.memzero(zrow[:])
            nc.scalar.dma_start(out=recv1[ZR1:ZR1 + 1, :], in_=zrow[:])
            for eli in range(EPC):
                nc.scalar.dma_start(
                    out=recv2[eli][R2[eli]:R2[eli] + 1, :], in_=zrow[:])

            # small tensors on the scalar queue; sync starts with x
            m2 = misc.tile([P, 2 * DC], BF16)
            nc.scalar.dma_start(out=m2[:], in_=maa2[:])
            s32 = misc.tile([P, T // P], I32)
            nc.scalar.dma_start(out=s32[:], in_=src32[:])
            sl16 = misc.tile([P, EPC * C // 16], I16)
            nc.scalar.dma_start(out=sl16[:], in_=slot16[:])
            s232 = misc.tile([P, EPC * C // P], I32)
            nc.scalar.dma_start(out=s232[:], in_=sl2_32[:])
            yg16 = {0: misc.tile([P, T // 16], I16, name="yg16_0s"),
                    1: misc.tile([P, T // 16], I16, name="yg16_1s")}
            nc.scalar.dma_start(out=yg16[0][:], in_=yg16_0d[:])
            nc.scalar.dma_start(out=yg16[1][:], in_=yg16_1d[:])

            ident = misc.tile([P, P], BF16)
            masks.make_identity(nc, ident[:])

            # receptance output stays resident in SBUF for phase D
            r_sb = misc.tile([P, T // P, D], BF16)

            wrt_sb = misc.tile([P, DC, D], BF16)
            wk_sb = {}
            wv_sb = {}

            def recept(hidx, xr_h, psr):
                """receptance for 512 tokens: r = sigmoid(xr @ wrt)"""
                for tt in range(4):
                    pr0 = psr.tile([P, 512], F32, space="PSUM", tag="pr0")
                    pr1 = psr.tile([P, 512], F32, space="PSUM", tag="pr1")
                    for dc in range(DC):
                        nc.tensor.matmul(
                            out=pr0[:],
                            lhsT=xr_h[:, dc, tt * P:(tt + 1) * P],
                            rhs=wrt_sb[:, dc, 0:512],
                            start=(dc == 0), stop=(dc == DC - 1))
                        nc.tensor.matmul(
                            out=pr1[:],
                            lhsT=xr_h[:, dc, tt * P:(tt + 1) * P],
                            rhs=wrt_sb[:, dc, 512:1024],
                            start=(dc == 0), stop=(dc == DC - 1))
                    ti = hidx * 4 + tt
                    nc.scalar.activation(out=r_sb[:, ti, 0:512], in_=pr0[:],
                                         func=AF.Sigmoid)
                    nc.scalar.activation(out=r_sb[:, ti, 512:1024],
                                         in_=pr1[:], func=AF.Sigmoid)

            with tc.tile_pool(name="pxr", bufs=3) as pxr, \
                 tc.tile_pool(name="psr", bufs=2, space="PSUM") as psr:
                xr_saved = {}
                # ---------------- phase A: token-shift mix in d-major
                with (
                    tc.tile_pool(name="pxb", bufs=2) as pxb,
                    tc.tile_pool(name="pdx", bufs=1) as pdx,
                    tc.tile_pool(name="pxk", bufs=1) as pxk,
                    tc.tile_pool(name="pxtm", bufs=4) as pxtm,
                    tc.tile_pool(name="pst", bufs=4, space="PSUM") as pst,
                ):
                    xbufs = []
                    for ck in range(2):
                        xb = pxb.tile([P, DC, 1025], BF16, tag="xb")
                        nc.sync.dma_start(
                            out=xb[:],
                            in_=x_t[:, :, ck * 1024:ck * 1024 + 1025])
                        xbufs.append(xb)
                        if ck == 0:
                            nc.sync.dma_start(
                                out=wrt_sb[:],
                                in_=wrt.rearrange("(c p) e -> p c e", p=P))
                    for eli in range(EPC):
                        wk_sb[eli] = pwk.tile([P, DC, F], BF16, tag="wk",
                                              name=f"wk_sb{eli}")
                        wv_sb[eli] = pwv.tile([P, FC, D], BF16, tag="wv",
                                              name=f"wv_sb{eli}")
                        if eli == 0:
                            nc.sync.dma_start(
                                out=wk_sb[0][:],
                                in_=wk[0].rearrange("(c p) f -> p c f", p=P))
                            nc.sync.dma_start(
                                out=wv_sb[0][:],
                                in_=wv[0].rearrange("(c p) f -> p c f", p=P))

                    for ck in range(2):
                        xb = xbufs[ck]
                        for h in range(2):
                            hidx = ck * 2 + h
                            o = h * 512
                            dx = pdx.tile([P, DC, 512], BF16, tag="dx")
                            # dxprev = xprev - x
                            nc.vector.tensor_sub(
                                out=dx[:], in0=xb[:, :, o:o + 512],
                                in1=xb[:, :, o + 1:o + 513])
                            xk_h = pxk.tile([P, DC, 512], BF16, tag="xk")
                            xr_h = pxr.tile([P, DC, 512], BF16, tag="xr")
                            for c in range(DC):
                                nc.vector.scalar_tensor_tensor(
                                    out=xk_h[:, c, :], in0=dx[:, c, :],
                                    scalar=m2[:, c:c + 1],
                                    in1=xb[:, c, o + 1:o + 513],
                                    op0=ALU.mult, op1=ALU.add)
                                nc.vector.scalar_tensor_tensor(
                                    out=xr_h[:, c, :], in0=dx[:, c, :],
                                    scalar=m2[:, DC + c:DC + c + 1],
                                    in1=xb[:, c, o + 1:o + 513],
                                    op0=ALU.mult, op1=ALU.add)
                            xr_saved[hidx] = xr_h
                            # transpose xk to token-major, scatter rows
                            # into the (half, parity) dispatch buffer
                            with tc.high_priority():
                                for tt in range(4):
                                    ps = pst.tile([P, DC, P], BF16,
                                                  space="PSUM", tag="ps")
                                    for c in range(DC):
                                        nc.tensor.transpose(
                                            out=ps[:, c, :],
                                            in_=xk_h[:, c,
                                                     tt * P:(tt + 1) * P],
                                            identity=ident[:])
                                    xtm = pxtm.tile([P, D], BF16, tag="xtm")
                                    nc.vector.tensor_copy(out=xtm[:],
                                                          in_=ps[:])
                                    t = hidx * 4 + tt
                                    g = ck * 2 + t % 2
                                    nc.gpsimd.indirect_dma_start(
                                        out=a1[g][:],
                                        out_offset=bass.IndirectOffsetOnAxis(
                                            ap=s32[:, t:t + 1], axis=0),
                                        in_=xtm[:], in_offset=None)
                        for g in (ck * 2, ck * 2 + 1):
                            nc.gpsimd.collective_compute(
                                "AllToAll", mybir.AluOpType.bypass,
                                replica_groups=rg,
                                ins=[a1[g][0:NCORES * dK[g], :]],
                                outs=[recv1[dbase[g]:dbase[g + 1], :]])
                        if ck == 0:
                            # receptance fills the PE while the first
                            # dispatch collectives are in flight
                            recept(0, xr_saved[0], psr)
                            recept(1, xr_saved[1], psr)

                # deferred receptance hides the second dispatch pair
                recept(2, xr_saved[2], psr)
                recept(3, xr_saved[3], psr)

            # ---------------- phase C: expert FFNs
            with (
                tc.tile_pool(name="pfx", bufs=2) as pfx,
                tc.tile_pool(name="pfh", bufs=1) as pfh,
                tc.tile_pool(name="phr", bufs=2) as phr,
                tc.tile_pool(name="pfy", bufs=3) as pfy,
                tc.tile_pool(name="psh", bufs=2, space="PSUM") as psh,
                tc.tile_pool(name="psy", bufs=2, space="PSUM") as psy,
                tc.tile_pool(name="pd", bufs=2) as pd,
                tc.tile_pool(name="pdo", bufs=1) as pdo,
            ):
                def phase_d(eli, accum):
                    """gather own tokens' y rows from recv2[eli] (other
                    tokens hit the zeroed trash row), multiply by r;
                    pass 0 writes out, pass 1 accumulates into it."""
                    for ck in range(T // 512):
                        yg = pd.tile([P, 4, D], BF16, tag="yg")
                        nc.gpsimd.dma_gather(
                            out_ap=yg[:], in_ap=recv2[eli][:],
                            idxs_ap=yg16[eli][:, ck * 32:(ck + 1) * 32],
                            num_idxs=512, num_idxs_reg=512, elem_size=D,
                            transpose=False)
                        yo = pdo.tile([P, 4, D], F32, tag="yo")
                        nc.vector.tensor_mul(
                            out=yo[:], in0=yg[:],
                            in1=r_sb[:, ck * 4:(ck + 1) * 4, :])
                        o_ap = out[ck * 512:(ck + 1) * 512, :].rearrange(
                            "(a p) d -> p a d", p=P)
                        if accum:
                            nc.gpsimd.dma_start(out=o_ap, in_=yo[:],
                                                accum_op=ALU.add)
                        else:
                            nc.scalar.dma_start(out=o_ap, in_=yo[:])

                for el in range(EPC):
                    if el == 1:
                        nc.sync.dma_start(
                            out=wk_sb[1][:],
                            in_=wk[1].rearrange("(c p) f -> p c f", p=P))
                        nc.sync.dma_start(
                            out=wv_sb[1][:],
                            in_=wv[1].rearrange("(c p) f -> p c f", p=P))
                    XTs = []
                    for ck in range(2):
                        XT = pfx.tile([P, DC, 512], BF16, tag="XT")
                        col0 = (el * C + ck * 512) // 16
                        nc.gpsimd.dma_gather(
                            out_ap=XT[:], in_ap=recv1[:],
                            idxs_ap=sl16[:, col0:col0 + 32],
                            num_idxs=512, num_idxs_reg=512, elem_size=D,
                            transpose=True)
                        XTs.append(XT)
                    for ck in range(2):
                        XT = XTs[ck]
                        ht = pfh.tile([P, FC, 512], BF16, tag="ht")
                        for ft in range(FC):
                            ph = psh.tile([P, 512], F32, space="PSUM", tag="ph")
                            for dc in range(DC):
                                nc.tensor.matmul(
                                    out=ph[:],
                                    lhsT=wk_sb[el][:, dc, ft * P:(ft + 1) * P],
                                    rhs=XT[:, dc, :],
                                    start=(dc == 0), stop=(dc == DC - 1))
                            hr = phr.tile([P, 512], BF16, tag="hr")
                            nc.scalar.activation(out=hr[:], in_=ph[:],
                                                 func=AF.Relu)
                            nc.vector.tensor_mul(out=ht[:, ft, :], in0=hr[:],
                                                 in1=hr[:])
                        for tt in range(4):
                            ysb = pfy.tile([P, D], BF16, tag="ysb")
                            py0 = psy.tile([P, 512], F32, space="PSUM", tag="py0")
                            py1 = psy.tile([P, 512], F32, space="PSUM", tag="py1")
                            for fc in range(FC):
                                nc.tensor.matmul(
                                    out=py0[:],
                                    lhsT=ht[:, fc, tt * P:(tt + 1) * P],
                                    rhs=wv_sb[el][:, fc, 0:512],
                                    start=(fc == 0), stop=(fc == FC - 1))
                                nc.tensor.matmul(
                                    out=py1[:],
                                    lhsT=ht[:, fc, tt * P:(tt + 1) * P],
                                    rhs=wv_sb[el][:, fc, 512:1024],
                                    start=(fc == 0), stop=(fc == FC - 1))
                            nc.vector.tensor_copy(out=ysb[:, 0:512], in_=py0[:])
                            nc.vector.tensor_copy(out=ysb[:, 512:1024],
                                                  in_=py1[:])
                            scol = el * (C // P) + ck * 4 + tt
                            par = (ck * 4 + tt) % 2
                            nc.gpsimd.indirect_dma_start(
                                out=a2[(el, par)][:],
                                out_offset=bass.IndirectOffsetOnAxis(
                                    ap=s232[:, scol:scol + 1], axis=0),
                                in_=ysb[:], in_offset=None)
                    Ke = NCORES * cK[(el, 0)]
                    Ko = NCORES * cK[(el, 1)]
                    nc.gpsimd.collective_compute(
                        "AllToAll", mybir.AluOpType.bypass, replica_groups=rg,
                        ins=[a2[(el, 0)][0:Ke, :]],
                        outs=[recv2[el][0:Ke, :]])
                    nc.gpsimd.collective_compute(
                        "AllToAll", mybir.AluOpType.bypass, replica_groups=rg,
                        ins=[a2[(el, 1)][0:Ko, :]],
                        outs=[recv2[el][Ke:Ke + Ko, :]])

                # ---------------- phase D: pass 0 runs during the second
                # combine collectives' flight, pass 1 accumulates after
                phase_d(0, accum=False)
                phase_d(1, accum=True)

    nc.finalize()
    return nc


def _prepare_inputs(x, token_ids, shift_state, time_maa_k, time_maa_r,
                    w_recept, w_key, w_value):
    cfg, idxs = _build_indices(token_ids)
    x = np.asarray(x, np.float32)
    shift = np.asarray(shift_state, np.float32)
    wrt = np.ascontiguousarray(np.asarray(w_recept, np.float32).T).astype(nbf16)
    wkb = np.asarray(w_key, np.float32).astype(nbf16)
    wvb = np.asarray(w_value, np.float32).astype(nbf16)
    mk = np.asarray(time_maa_k, np.float32)
    mr = np.asarray(time_maa_r, np.float32)
    # [P, 2*DC]: col c = maa_k[c*128+p], col DC+c = maa_r[c*128+p]
    maa2 = np.concatenate(
        [mk.reshape(DC, P).T, mr.reshape(DC, P).T], axis=1).astype(nbf16)
    maa2 = np.ascontiguousarray(maa2)

    in_maps = []
    for k in range(NCORES):
        # x transposed, d-major: x_t[p, c, 1+t] = x[k, t, c*128+p]
        xk_full = np.concatenate([shift[k][:, None], x[k].T], axis=1)
        x_t = np.ascontiguousarray(
            xk_full.reshape(DC, P, T + 1).transpose(1, 0, 2)).astype(nbf16)
        in_maps.append({
            "x_t": x_t,
            "maa2": maa2, "wrt": wrt,
            "wk": np.ascontiguousarray(wkb[EPC * k:EPC * (k + 1)]),
            "wv": np.ascontiguousarray(wvb[EPC * k:EPC * (k + 1)]),
            **idxs[k],
        })
    return cfg, in_maps


def kernel(x, token_ids, shift_state, time_maa_k, time_maa_r,
           w_recept, w_key, w_value, _trace=False):
    cfg, in_maps = _prepare_inputs(x, token_ids, shift_state, time_maa_k,
                                   time_maa_r, w_recept, w_key, w_value)
    if cfg not in _CACHE:
        _CACHE[cfg] = _build_nc(cfg)
    nc = _CACHE[cfg]
    res = run_bass_kernel_spmd(nc, in_maps, core_ids=list(range(NCORES)),
                               trace=_trace)
    kernel.last_result = res
    y = np.stack([res.results[k]["out"] for k in range(NCORES)], axis=0)
    return y.astype(np.float32)


# revision 25
# speedup vs baseline: 1.2235x; 1.2235x over previous
"""Expert-parallel CMoE kernel for 8 Trainium2 NeuronCores.

Sharding (hardcoded for B=8, T=2048, D=1024, F=2048, E=16, C=1024):
  core k owns batch k (token shift, receptance, output) and experts
  {2k, 2k+1} (FFN). Hash routing is int math on token_ids, done on host;
  the resulting permutations ship to the cores as index tensors.

v6 design:
  - x ships transposed (d-major, bf16) with the shift state as column 0,
    so the token-shift mix is a free-axis slice: one DMA read of x.
  - xk/xr computed in d-major; receptance consumes xrT from SBUF and r
    stays resident in SBUF for phase D.
  - xk is PE-transposed back to token-major for the dispatch scatter.
  - consecutive indirect scatters to one DRAM tile serialize on DMA
    completion (~8us/hop), so dispatch uses FOUR buffers (half x tile
    parity) and combine TWO per expert (slot-tile parity): chains halve
    and the smaller AllToAlls pipeline on the CC core.
  - receptance matmuls hide the dispatch collectives (2 run after the
    first pair fires, 2 after the second pair).
  - phase C per expert: transposing dma_gather -> FFN1 -> relu^2 ->
    FFN2 -> indirect scatter (parity-split) -> two combine AllToAlls.
  - phase D runs twice in natural token order: pass 0 (expert-parity 0,
    other tokens hit the zeroed trash row) lands during the expert-1
    combine collectives; pass 1 DMA-accumulates into out after them.
All matmuls bf16 with fp32 PSUM accumulation; dropped tokens and empty
expert slots route through zeroed trash rows.
"""
import sys

for _p in ("/opt/trn_rl_repo", "/root/.axon_site/_ro/trn_rl_repo"):
    if _p not in sys.path:
        sys.path.append(_p)

import numpy as np
import ml_dtypes

import concourse.bass as bass
import concourse.bacc as bacc
import concourse.mybir as mybir
import concourse.tile as tile
from concourse import masks
from concourse.bass_utils import run_bass_kernel_spmd

P = 128
B, T, D, F, E = 8, 2048, 1024, 2048, 16
N = B * T
C = max(4, N // E)          # 1024
HASH_PRIME = 5099
NCORES = 8
EPC = E // NCORES           # experts per core = 2
DC = D // P                 # 8
FC = F // P                 # 16
BF16 = mybir.dt.bfloat16
F32 = mybir.dt.float32
I16 = mybir.dt.int16
I32 = mybir.dt.int32
nbf16 = ml_dtypes.bfloat16
AF = mybir.ActivationFunctionType
ALU = mybir.AluOpType

_CACHE = {}


def _r16(v):
    return int(-(-int(v) // 16) * 16)


def _wrap16(a):
    a = np.asarray(a, np.int16)
    w = a.reshape(-1, 16).T.copy()       # j at [j%16, j//16]
    return np.tile(w, (8, 1))            # replicated across 8 Q7 cores


def _route(token_ids):
    tid = np.asarray(token_ids).reshape(N).astype(np.int64)
    e = (tid * HASH_PRIME) % E
    onehot = (e[:, None] == np.arange(E)).astype(np.int64)
    pos = onehot.cumsum(0)[np.arange(N), e] - 1
    keep = pos < C
    return e, pos, keep


def _build_indices(token_ids):
    e, pos, keep = _route(token_ids)
    src = np.arange(N) // T
    local_t = np.arange(N) % T
    dst = e // EPC
    el = e % EPC
    tile_i = local_t // P

    def pack(mask):
        rank = np.zeros(N, np.int64)
        cnt = np.zeros((NCORES, NCORES), np.int64)
        for n in np.nonzero(mask)[0]:
            rank[n] = cnt[src[n], dst[n]]
            cnt[src[n], dst[n]] += 1
        return rank, _r16(cnt.max())

    # ---- dispatch: 4 groups = (half, tile parity); the first pair of
    # collectives fires mid-phase-A, chains per buffer are 4 hops
    dgrp = (local_t >= T // 2) * 2 + (tile_i % 2)
    dK = []
    drank = np.zeros(N, np.int64)
    for g in range(4):
        m = keep & (dgrp == g)
        r_, K_ = pack(m)
        dK.append(K_)
        drank[m] = r_[m]
    srcoff = np.zeros(N, np.int64)
    for g in range(4):
        m = dgrp == g
        srcoff[m & keep] = dst[m & keep] * dK[g] + drank[m & keep]
        srcoff[m & ~keep] = NCORES * dK[g]
    dbase = [0]
    for g in range(4):
        dbase.append(dbase[-1] + NCORES * dK[g])
    ZR1 = dbase[4]
    recv_row = np.full((NCORES, EPC * C), ZR1, np.int64)
    for n in np.nonzero(keep)[0]:
        recv_row[dst[n], el[n] * C + pos[n]] = \
            dbase[dgrp[n]] + src[n] * dK[dgrp[n]] + drank[n]

    # ---- combine: one chunk per expert parity; note sender is the
    # expert owner (dst), receiver the token owner (src)
    rank_0, K0 = pack(keep & (el == 0))
    rank_1, K1 = pack(keep & (el == 1))
    ZROW = NCORES * (K0 + K1)
    sl2 = np.zeros((NCORES, EPC, C), np.int64)
    sl2[:, 0, :] = NCORES * K0
    sl2[:, 1, :] = NCORES * K1
    ygather = np.full(N, ZROW, np.int64)
    for n in range(N):
        if not keep[n]:
            continue
        if el[n] == 0:
            sl2[dst[n], 0, pos[n]] = src[n] * K0 + rank_0[n]
            ygather[n] = dst[n] * K0 + rank_0[n]
        else:
            sl2[dst[n], 1, pos[n]] = src[n] * K1 + rank_1[n]
            ygather[n] = NCORES * K0 + dst[n] * K1 + rank_1[n]

    per_core = []
    for k in range(NCORES):
        tok = slice(k * T, (k + 1) * T)
        per_core.append({
            "src32": srcoff[tok].astype(np.int32).reshape(T // P, P).T.copy(),
            "slot16": _wrap16(recv_row[k]),
            "sl2_32": np.concatenate(
                [sl2[k, 0].reshape(-1, P).T, sl2[k, 1].reshape(-1, P).T],
                axis=1).astype(np.int32).copy(),
            "ygather16": _wrap16(ygather[tok]),
        })
    return (tuple(dK), K0, K1), per_core


def _build_nc(cfg):
    dK, K0, K1 = cfg
    dK = list(dK)
    dbase = [0]
    for g in range(4):
        dbase.append(dbase[-1] + NCORES * dK[g])
    ZR1 = dbase[4]
    K2 = {0: K0, 1: K1}
    off2 = {0: 0, 1: NCORES * K0}
    R2 = NCORES * (K0 + K1)
    nc = bacc.Bacc("TRN2", target_bir_lowering=False, debug=False,
                   num_devices=NCORES)

    x_t = nc.dram_tensor("x_t", [P, DC, T + 1], BF16, kind="ExternalInput")
    maa2 = nc.dram_tensor("maa2", [P, 2 * DC], BF16, kind="ExternalInput")
    wrt = nc.dram_tensor("wrt", [D, D], BF16, kind="ExternalInput")
    wk = nc.dram_tensor("wk", [EPC, D, F], BF16, kind="ExternalInput")
    wv = nc.dram_tensor("wv", [EPC, F, D], BF16, kind="ExternalInput")
    src32 = nc.dram_tensor("src32", [P, T // P], I32, kind="ExternalInput")
    slot16 = nc.dram_tensor("slot16", [P, EPC * C // 16], I16,
                            kind="ExternalInput")
    sl2_32 = nc.dram_tensor("sl2_32", [P, EPC * C // P], I32,
                            kind="ExternalInput")
    ygather16 = nc.dram_tensor("ygather16", [P, T // 16], I16,
                               kind="ExternalInput")
    out = nc.dram_tensor("out", [T, D], F32, kind="ExternalOutput")

    rg = [list(range(NCORES))]

    with tile.TileContext(nc) as tc:
        with (
            tc.tile_pool(name="dram", bufs=1, space="DRAM") as dram,
            tc.tile_pool(name="misc", bufs=1) as misc,
            tc.tile_pool(name="pwk", bufs=1) as pwk,
            tc.tile_pool(name="pwv", bufs=1) as pwv,
        ):
            a1 = {g: dram.tile([NCORES * dK[g] + 1, D], BF16,
                               name=f"a1_{g}") for g in range(4)}
            recv1 = dram.tile([ZR1 + 1, D], BF16)
            a2 = {eli: dram.tile([NCORES * K2[eli] + 1, D], BF16,
                                 name=f"a2_{eli}") for eli in range(EPC)}
            recv2 = dram.tile([R2 + 1, D], BF16)

            zrow = misc.tile([1, D], BF16)
            nc.vector.memzero(zrow[:])
            nc.scalar.dma_start(out=recv1[ZR1:ZR1 + 1, :], in_=zrow[:])
            nc.scalar.dma_start(out=recv2[R2:R2 + 1, :], in_=zrow[:])

            # small tensors on the scalar queue; sync starts with x
            m2 = misc.tile([P, 2 * DC], BF16)
            nc.scalar.dma_start(out=m2[:], in_=maa2[:])
            s32 = misc.tile([P, T // P], I32)
            nc.scalar.dma_start(out=s32[:], in_=src32[:])
            sl16 = misc.tile([P, EPC * C // 16], I16)
            nc.scalar.dma_start(out=sl16[:], in_=slot16[:])
            s232 = misc.tile([P, EPC * C // P], I32)
            nc.scalar.dma_start(out=s232[:], in_=sl2_32[:])
            yg16 = misc.tile([P, T // 16], I16)
            nc.scalar.dma_start(out=yg16[:], in_=ygather16[:])

            ident = misc.tile([P, P], BF16)
            masks.make_identity(nc, ident[:])

            # receptance output stays resident in SBUF for phase D
            r_sb = misc.tile([P, T // P, D], BF16)

            wrt_sb = misc.tile([P, DC, D], BF16)
            wk_sb = {}
            wv_sb = {}

            def recept(hidx, xr_h, psr):
                """receptance for 512 tokens: r = sigmoid(xr @ wrt)"""
                for tt in range(4):
                    pr0 = psr.tile([P, 512], F32, space="PSUM", tag="pr0")
                    pr1 = psr.tile([P, 512], F32, space="PSUM", tag="pr1")
                    for dc in range(DC):
                        nc.tensor.matmul(
                            out=pr0[:],
                            lhsT=xr_h[:, dc, tt * P:(tt + 1) * P],
                            rhs=wrt_sb[:, dc, 0:512],
                            start=(dc == 0), stop=(dc == DC - 1))
                        nc.tensor.matmul(
                            out=pr1[:],
                            lhsT=xr_h[:, dc, tt * P:(tt + 1) * P],
                            rhs=wrt_sb[:, dc, 512:1024],
                            start=(dc == 0), stop=(dc == DC - 1))
                    ti = hidx * 4 + tt
                    nc.scalar.activation(out=r_sb[:, ti, 0:512], in_=pr0[:],
                                         func=AF.Sigmoid)
                    nc.scalar.activation(out=r_sb[:, ti, 512:1024],
                                         in_=pr1[:], func=AF.Sigmoid)

            with tc.tile_pool(name="pxr", bufs=3) as pxr, \
                 tc.tile_pool(name="psr", bufs=2, space="PSUM") as psr:
                xr_saved = {}
                # ---------------- phase A: token-shift mix in d-major
                with (
                    tc.tile_pool(name="pxb", bufs=2) as pxb,
                    tc.tile_pool(name="pdx", bufs=1) as pdx,
                    tc.tile_pool(name="pxk", bufs=1) as pxk,
                    tc.tile_pool(name="pxtm", bufs=4) as pxtm,
                    tc.tile_pool(name="pst", bufs=4, space="PSUM") as pst,
                ):
                    xbufs = []
                    for ck in range(2):
                        xb = pxb.tile([P, DC, 1025], BF16, tag="xb")
                        nc.sync.dma_start(
                            out=xb[:],
                            in_=x_t[:, :, ck * 1024:ck * 1024 + 1025])
                        xbufs.append(xb)
                        if ck == 0:
                            nc.sync.dma_start(
                                out=wrt_sb[:],
                                in_=wrt.rearrange("(c p) e -> p c e", p=P))
                    for eli in range(EPC):
                        wk_sb[eli] = pwk.tile([P, DC, F], BF16, tag="wk",
                                              name=f"wk_sb{eli}")
                        wv_sb[eli] = pwv.tile([P, FC, D], BF16, tag="wv",
                                              name=f"wv_sb{eli}")
                        if eli == 0:
                            nc.sync.dma_start(
                                out=wk_sb[0][:],
                                in_=wk[0].rearrange("(c p) f -> p c f", p=P))
                            nc.sync.dma_start(
                                out=wv_sb[0][:],
                                in_=wv[0].rearrange("(c p) f -> p c f", p=P))

                    for ck in range(2):
                        xb = xbufs[ck]
                        for h in range(2):
                            hidx = ck * 2 + h
                            o = h * 512
                            dx = pdx.tile([P, DC, 512], BF16, tag="dx")
                            # dxprev = xprev - x
                            nc.vector.tensor_sub(
                                out=dx[:], in0=xb[:, :, o:o + 512],
                                in1=xb[:, :, o + 1:o + 513])
                            xk_h = pxk.tile([P, DC, 512], BF16, tag="xk")
                            xr_h = pxr.tile([P, DC, 512], BF16, tag="xr")
                            for c in range(DC):
                                nc.vector.scalar_tensor_tensor(
                                    out=xk_h[:, c, :], in0=dx[:, c, :],
                                    scalar=m2[:, c:c + 1],
                                    in1=xb[:, c, o + 1:o + 513],
                                    op0=ALU.mult, op1=ALU.add)
                                nc.vector.scalar_tensor_tensor(
                                    out=xr_h[:, c, :], in0=dx[:, c, :],
                                    scalar=m2[:, DC + c:DC + c + 1],
                                    in1=xb[:, c, o + 1:o + 513],
                                    op0=ALU.mult, op1=ALU.add)
                            xr_saved[hidx] = xr_h
                            # transpose xk to token-major, scatter rows
                            # into the (half, parity) dispatch buffer
                            with tc.high_priority():
                                for tt in range(4):
                                    ps = pst.tile([P, DC, P], BF16,
                                                  space="PSUM", tag="ps")
                                    for c in range(DC):
                                        nc.tensor.transpose(
                                            out=ps[:, c, :],
                                            in_=xk_h[:, c,
                                                     tt * P:(tt + 1) * P],
                                            identity=ident[:])
                                    xtm = pxtm.tile([P, D], BF16, tag="xtm")
                                    nc.vector.tensor_copy(out=xtm[:],
                                                          in_=ps[:])
                                    t = hidx * 4 + tt
                                    g = ck * 2 + t % 2
                                    nc.gpsimd.indirect_dma_start(
                                        out=a1[g][:],
                                        out_offset=bass.IndirectOffsetOnAxis(
                                            ap=s32[:, t:t + 1], axis=0),
                                        in_=xtm[:], in_offset=None)
                        for g in (ck * 2, ck * 2 + 1):
                            nc.gpsimd.collective_compute(
                                "AllToAll", mybir.AluOpType.bypass,
                                replica_groups=rg,
                                ins=[a1[g][0:NCORES * dK[g], :]],
                                outs=[recv1[dbase[g]:dbase[g + 1], :]])
                        if ck == 0:
                            # receptance fills the PE while the first
                            # dispatch collectives are in flight
                            recept(0, xr_saved[0], psr)
                            recept(1, xr_saved[1], psr)

                # deferred receptance hides the second dispatch pair
                recept(2, xr_saved[2], psr)
                recept(3, xr_saved[3], psr)

            # ---------------- phase C: expert FFNs
            with (
                tc.tile_pool(name="pfx", bufs=2) as pfx,
                tc.tile_pool(name="pfh", bufs=1) as pfh,
                tc.tile_pool(name="phr", bufs=2) as phr,
                tc.tile_pool(name="pfy", bufs=2) as pfy,
                tc.tile_pool(name="psh", bufs=2, space="PSUM") as psh,
                tc.tile_pool(name="psy", bufs=2, space="PSUM") as psy,
                tc.tile_pool(name="pd", bufs=2) as pd,
                tc.tile_pool(name="pdo", bufs=2) as pdo,
            ):
                def phase_d():
                    """gather own tokens' y rows, multiply by r"""
                    for ck in range(T // 512):
                        yg = pd.tile([P, 4, D], BF16, tag="yg")
                        nc.gpsimd.dma_gather(
                            out_ap=yg[:], in_ap=recv2[:],
                            idxs_ap=yg16[:, ck * 32:(ck + 1) * 32],
                            num_idxs=512, num_idxs_reg=512, elem_size=D,
                            transpose=False)
                        yo = pdo.tile([P, 4, D], F32, tag="yo")
                        nc.vector.tensor_mul(
                            out=yo[:], in0=yg[:],
                            in1=r_sb[:, ck * 4:(ck + 1) * 4, :])
                        nc.scalar.dma_start(
                            out=out[ck * 512:(ck + 1) * 512, :].rearrange(
                                "(a p) d -> p a d", p=P),
                            in_=yo[:])

                for el in range(EPC):
                    if el == 1:
                        nc.sync.dma_start(
                            out=wk_sb[1][:],
                            in_=wk[1].rearrange("(c p) f -> p c f", p=P))
                        nc.sync.dma_start(
                            out=wv_sb[1][:],
                            in_=wv[1].rearrange("(c p) f -> p c f", p=P))
                    XTs = []
                    for ck in range(2):
                        XT = pfx.tile([P, DC, 512], BF16, tag="XT")
                        col0 = (el * C + ck * 512) // 16
                        nc.gpsimd.dma_gather(
                            out_ap=XT[:], in_ap=recv1[:],
                            idxs_ap=sl16[:, col0:col0 + 32],
                            num_idxs=512, num_idxs_reg=512, elem_size=D,
                            transpose=True)
                        XTs.append(XT)
                    for ck in range(2):
                        XT = XTs[ck]
                        ht = pfh.tile([P, FC, 512], BF16, tag="ht")
                        for ft in range(FC):
                            ph = psh.tile([P, 512], F32, space="PSUM", tag="ph")
                            for dc in range(DC):
                                nc.tensor.matmul(
                                    out=ph[:],
                                    lhsT=wk_sb[el][:, dc, ft * P:(ft + 1) * P],
                                    rhs=XT[:, dc, :],
                                    start=(dc == 0), stop=(dc == DC - 1))
                            hr = phr.tile([P, 512], BF16, tag="hr")
                            nc.scalar.activation(out=hr[:], in_=ph[:],
                                                 func=AF.Relu)
                            nc.vector.tensor_mul(out=ht[:, ft, :], in0=hr[:],
                                                 in1=hr[:])
                        for tt in range(4):
                            ysb = pfy.tile([P, D], BF16, tag="ysb")
                            py0 = psy.tile([P, 512], F32, space="PSUM", tag="py0")
                            py1 = psy.tile([P, 512], F32, space="PSUM", tag="py1")
                            for fc in range(FC):
                                nc.tensor.matmul(
                                    out=py0[:],
                                    lhsT=ht[:, fc, tt * P:(tt + 1) * P],
                                    rhs=wv_sb[el][:, fc, 0:512],
                                    start=(fc == 0), stop=(fc == FC - 1))
                                nc.tensor.matmul(
                                    out=py1[:],
                                    lhsT=ht[:, fc, tt * P:(tt + 1) * P],
                                    rhs=wv_sb[el][:, fc, 512:1024],
                                    start=(fc == 0), stop=(fc == FC - 1))
                            nc.vector.tensor_copy(out=ysb[:, 0:512], in_=py0[:])
                            nc.vector.tensor_copy(out=ysb[:, 512:1024],
                                                  in_=py1[:])
                            scol = el * (C // P) + ck * 4 + tt
                            nc.gpsimd.indirect_dma_start(
                                out=a2[el][:],
                                out_offset=bass.IndirectOffsetOnAxis(
                                    ap=s232[:, scol:scol + 1], axis=0),
                                in_=ysb[:], in_offset=None)
                    nc.gpsimd.collective_compute(
                        "AllToAll", mybir.AluOpType.bypass, replica_groups=rg,
                        ins=[a2[el][0:NCORES * K2[el], :]],
                        outs=[recv2[off2[el]:off2[el] + NCORES * K2[el], :]])

                # ---------------- phase D: gather own rows, multiply by r
                phase_d()

    nc.finalize()
    return nc


def _prepare_inputs(x, token_ids, shift_state, time_maa_k, time_maa_r,
                    w_recept, w_key, w_value):
    cfg, idxs = _build_indices(token_ids)
    x = np.asarray(x, np.float32)
    shift = np.asarray(shift_state, np.float32)
    wrt = np.ascontiguousarray(np.asarray(w_recept, np.float32).T).astype(nbf16)
    wkb = np.asarray(w_key, np.float32).astype(nbf16)
    wvb = np.asarray(w_value, np.float32).astype(nbf16)
    mk = np.asarray(time_maa_k, np.float32)
    mr = np.asarray(time_maa_r, np.float32)
    # [P, 2*DC]: col c = maa_k[c*128+p], col DC+c = maa_r[c*128+p]
    maa2 = np.concatenate(
        [mk.reshape(DC, P).T, mr.reshape(DC, P).T], axis=1).astype(nbf16)
    maa2 = np.ascontiguousarray(maa2)

    in_maps = []
    for k in range(NCORES):
        # x transposed, d-major: x_t[p, c, 1+t] = x[k, t, c*128+p]
        xk_full = np.concatenate([shift[k][:, None], x[k].T], axis=1)
        x_t = np.ascontiguousarray(
            xk_full.reshape(DC, P, T + 1).transpose(1, 0, 2)).astype(nbf16)
        in_maps.append({
            "x_t": x_t,
            "maa2": maa2, "wrt": wrt,
            "wk": np.ascontiguousarray(wkb[EPC * k:EPC * (k + 1)]),
            "wv": np.ascontiguousarray(wvb[EPC * k:EPC * (k + 1)]),
            **idxs[k],
        })
    return cfg, in_maps


def kernel(x, token_ids, shift_state, time_maa_k, time_maa_r,
           w_recept, w_key, w_value, _trace=False, _trace_cores=None):
    cfg, in_maps = _prepare_inputs(x, token_ids, shift_state, time_maa_k,
                                   time_maa_r, w_recept, w_key, w_value)
    if cfg not in _CACHE:
        _CACHE[cfg] = _build_nc(cfg)
    nc = _CACHE[cfg]
    res = run_bass_kernel_spmd(nc, in_maps, core_ids=list(range(NCORES)),
                               trace=_trace, trace_cores=_trace_cores)
    kernel.last_result = res
    y = np.stack([res.results[k]["out"] for k in range(NCORES)], axis=0)
    return y.astype(np.float32)


# revision 30
# speedup vs baseline: 1.2556x; 1.0263x over previous
"""Expert-parallel CMoE kernel for 8 Trainium2 NeuronCores.

Sharding (hardcoded for B=8, T=2048, D=1024, F=2048, E=16, C=1024):
  core k owns batch k (token shift, receptance, output) and experts
  {2k, 2k+1} (FFN). Hash routing is int math on token_ids, done on host;
  the resulting permutations ship to the cores as index tensors.

v6 design:
  - x ships transposed (d-major, bf16) with the shift state as column 0,
    so the token-shift mix is a free-axis slice: one DMA read of x.
  - xk/xr computed in d-major; receptance consumes xrT from SBUF and r
    stays resident in SBUF for phase D.
  - xk is PE-transposed back to token-major for the dispatch scatter.
  - consecutive indirect scatters to one DRAM tile serialize on DMA
    completion (~8us/hop), so dispatch uses FOUR buffers (half x tile
    parity) and combine TWO per expert (slot-tile parity): chains halve
    and the smaller AllToAlls pipeline on the CC core.
  - receptance matmuls hide the dispatch collectives (2 run after the
    first pair fires, 2 after the second pair).
  - phase C per expert: transposing dma_gather -> FFN1 -> relu^2 ->
    FFN2 -> indirect scatter (parity-split) -> two combine AllToAlls.
  - phase D runs twice in natural token order: pass 0 (expert-parity 0,
    other tokens hit the zeroed trash row) lands during the expert-1
    combine collectives; pass 1 DMA-accumulates into out after them.
All matmuls bf16 with fp32 PSUM accumulation; dropped tokens and empty
expert slots route through zeroed trash rows.
"""
import sys

for _p in ("/opt/trn_rl_repo", "/root/.axon_site/_ro/trn_rl_repo"):
    if _p not in sys.path:
        sys.path.append(_p)

import numpy as np
import ml_dtypes

import concourse.bass as bass
import concourse.bacc as bacc
import concourse.mybir as mybir
import concourse.tile as tile
from concourse import masks
from concourse.bass_utils import run_bass_kernel_spmd

P = 128
B, T, D, F, E = 8, 2048, 1024, 2048, 16
N = B * T
C = max(4, N // E)          # 1024
HASH_PRIME = 5099
NCORES = 8
EPC = E // NCORES           # experts per core = 2
DC = D // P                 # 8
FC = F // P                 # 16
BF16 = mybir.dt.bfloat16
F32 = mybir.dt.float32
I16 = mybir.dt.int16
I32 = mybir.dt.int32
nbf16 = ml_dtypes.bfloat16
AF = mybir.ActivationFunctionType
ALU = mybir.AluOpType

_CACHE = {}


def _r16(v):
    return int(-(-int(v) // 16) * 16)


def _wrap16(a):
    a = np.asarray(a, np.int16)
    w = a.reshape(-1, 16).T.copy()       # j at [j%16, j//16]
    return np.tile(w, (8, 1))            # replicated across 8 Q7 cores


def _route(token_ids):
    tid = np.asarray(token_ids).reshape(N).astype(np.int64)
    e = (tid * HASH_PRIME) % E
    onehot = (e[:, None] == np.arange(E)).astype(np.int64)
    pos = onehot.cumsum(0)[np.arange(N), e] - 1
    keep = pos < C
    return e, pos, keep


def _build_indices(token_ids):
    e, pos, keep = _route(token_ids)
    src = np.arange(N) // T
    local_t = np.arange(N) % T
    dst = e // EPC
    el = e % EPC
    tile_i = local_t // P

    def pack(mask):
        rank = np.zeros(N, np.int64)
        cnt = np.zeros((NCORES, NCORES), np.int64)
        for n in np.nonzero(mask)[0]:
            rank[n] = cnt[src[n], dst[n]]
            cnt[src[n], dst[n]] += 1
        return rank, _r16(cnt.max())

    # ---- dispatch: 4 groups = (half, tile parity); the first pair of
    # collectives fires mid-phase-A, chains per buffer are 4 hops
    dgrp = (local_t >= T // 2) * 2 + (tile_i % 2)
    dK = []
    drank = np.zeros(N, np.int64)
    for g in range(4):
        m = keep & (dgrp == g)
        r_, K_ = pack(m)
        dK.append(K_)
        drank[m] = r_[m]
    srcoff = np.zeros(N, np.int64)
    for g in range(4):
        m = dgrp == g
        srcoff[m & keep] = dst[m & keep] * dK[g] + drank[m & keep]
        srcoff[m & ~keep] = NCORES * dK[g]
    dbase = [0]
    for g in range(4):
        dbase.append(dbase[-1] + NCORES * dK[g])
    ZR1 = dbase[4]

    # per-expert slot reorder: first-half (groups 0/1) tokens first, so
    # the expert's first 384 slots only need the first two collectives
    # and its FFN can start before the second pair lands
    npos = np.full(N, -1, np.int64)
    for d in range(NCORES):
        for eli in range(EPC):
            m = keep & (dst == d) & (el == eli)
            a_n = np.nonzero(m & (dgrp < 2))[0]
            b_n = np.nonzero(m & (dgrp >= 2))[0]
            assert len(a_n) >= 384, f"a-half underflow: {len(a_n)}"
            order = np.concatenate([a_n, b_n])
            npos[order] = np.arange(len(order))

    recv_row = np.full((NCORES, EPC * C), ZR1, np.int64)
    for n in np.nonzero(keep)[0]:
        recv_row[dst[n], el[n] * C + npos[n]] = \
            dbase[dgrp[n]] + src[n] * dK[dgrp[n]] + drank[n]

    # ---- combine: one chunk per expert parity; note sender is the
    # expert owner (dst), receiver the token owner (src)
    rank_0, K0 = pack(keep & (el == 0))
    rank_1, K1 = pack(keep & (el == 1))
    ZROW = NCORES * (K0 + K1)
    sl2 = np.zeros((NCORES, EPC, C), np.int64)
    sl2[:, 0, :] = NCORES * K0
    sl2[:, 1, :] = NCORES * K1
    ygather = np.full(N, ZROW, np.int64)
    for n in range(N):
        if not keep[n]:
            continue
        if el[n] == 0:
            sl2[dst[n], 0, npos[n]] = src[n] * K0 + rank_0[n]
            ygather[n] = dst[n] * K0 + rank_0[n]
        else:
            sl2[dst[n], 1, npos[n]] = src[n] * K1 + rank_1[n]
            ygather[n] = NCORES * K0 + dst[n] * K1 + rank_1[n]

    per_core = []
    for k in range(NCORES):
        tok = slice(k * T, (k + 1) * T)
        per_core.append({
            "src32": srcoff[tok].astype(np.int32).reshape(T // P, P).T.copy(),
            "slot16": _wrap16(recv_row[k]),
            "sl2_32": np.concatenate(
                [sl2[k, 0].reshape(-1, P).T, sl2[k, 1].reshape(-1, P).T],
                axis=1).astype(np.int32).copy(),
            "ygather16": _wrap16(ygather[tok]),
        })
    return (tuple(dK), K0, K1), per_core


def _build_nc(cfg):
    dK, K0, K1 = cfg
    dK = list(dK)
    dbase = [0]
    for g in range(4):
        dbase.append(dbase[-1] + NCORES * dK[g])
    ZR1 = dbase[4]
    K2 = {0: K0, 1: K1}
    off2 = {0: 0, 1: NCORES * K0}
    R2 = NCORES * (K0 + K1)
    nc = bacc.Bacc("TRN2", target_bir_lowering=False, debug=False,
                   num_devices=NCORES)

    x_t = nc.dram_tensor("x_t", [P, DC, T + 1], BF16, kind="ExternalInput")
    maa2 = nc.dram_tensor("maa2", [P, 2 * DC], BF16, kind="ExternalInput")
    wrt = nc.dram_tensor("wrt", [D, D], BF16, kind="ExternalInput")
    wk = nc.dram_tensor("wk", [EPC, D, F], BF16, kind="ExternalInput")
    wv = nc.dram_tensor("wv", [EPC, F, D], BF16, kind="ExternalInput")
    src32 = nc.dram_tensor("src32", [P, T // P], I32, kind="ExternalInput")
    slot16 = nc.dram_tensor("slot16", [P, EPC * C // 16], I16,
                            kind="ExternalInput")
    sl2_32 = nc.dram_tensor("sl2_32", [P, EPC * C // P], I32,
                            kind="ExternalInput")
    ygather16 = nc.dram_tensor("ygather16", [P, T // 16], I16,
                               kind="ExternalInput")
    out = nc.dram_tensor("out", [T, D], F32, kind="ExternalOutput")

    rg = [list(range(NCORES))]

    with tile.TileContext(nc) as tc:
        with (
            tc.tile_pool(name="dram", bufs=1, space="DRAM") as dram,
            tc.tile_pool(name="misc", bufs=1) as misc,
            tc.tile_pool(name="pwk", bufs=1) as pwk,
            tc.tile_pool(name="pwv", bufs=1) as pwv,
        ):
            a1 = {g: dram.tile([NCORES * dK[g] + 1, D], BF16,
                               name=f"a1_{g}") for g in range(4)}
            recv1 = dram.tile([ZR1 + 1, D], BF16)
            a2 = {eli: dram.tile([NCORES * K2[eli] + 1, D], BF16,
                                 name=f"a2_{eli}") for eli in range(EPC)}
            recv2 = dram.tile([R2 + 1, D], BF16)

            zrow = misc.tile([1, D], BF16)
            nc.vector.memzero(zrow[:])
            nc.scalar.dma_start(out=recv1[ZR1:ZR1 + 1, :], in_=zrow[:])
            nc.scalar.dma_start(out=recv2[R2:R2 + 1, :], in_=zrow[:])

            # small tensors on the scalar queue; sync starts with x
            m2 = misc.tile([P, 2 * DC], BF16)
            nc.scalar.dma_start(out=m2[:], in_=maa2[:])
            s32 = misc.tile([P, T // P], I32)
            nc.scalar.dma_start(out=s32[:], in_=src32[:])
            sl16 = misc.tile([P, EPC * C // 16], I16)
            nc.scalar.dma_start(out=sl16[:], in_=slot16[:])
            s232 = misc.tile([P, EPC * C // P], I32)
            nc.scalar.dma_start(out=s232[:], in_=sl2_32[:])
            yg16 = misc.tile([P, T // 16], I16)
            nc.scalar.dma_start(out=yg16[:], in_=ygather16[:])

            ident = misc.tile([P, P], BF16)
            masks.make_identity(nc, ident[:])

            # receptance output stays resident in SBUF for phase D
            r_sb = misc.tile([P, T // P, D], BF16)

            wrt_sb = misc.tile([P, DC, D], BF16)
            wk_sb = {}
            wv_sb = {}

            def recept(hidx, xr_h, psr):
                """receptance for 512 tokens: r = sigmoid(xr @ wrt)"""
                for tt in range(4):
                    pr0 = psr.tile([P, 512], F32, space="PSUM", tag="pr0")
                    pr1 = psr.tile([P, 512], F32, space="PSUM", tag="pr1")
                    for dc in range(DC):
                        nc.tensor.matmul(
                            out=pr0[:],
                            lhsT=xr_h[:, dc, tt * P:(tt + 1) * P],
                            rhs=wrt_sb[:, dc, 0:512],
                            start=(dc == 0), stop=(dc == DC - 1))
                        nc.tensor.matmul(
                            out=pr1[:],
                            lhsT=xr_h[:, dc, tt * P:(tt + 1) * P],
                            rhs=wrt_sb[:, dc, 512:1024],
                            start=(dc == 0), stop=(dc == DC - 1))
                    ti = hidx * 4 + tt
                    nc.scalar.activation(out=r_sb[:, ti, 0:512], in_=pr0[:],
                                         func=AF.Sigmoid)
                    nc.scalar.activation(out=r_sb[:, ti, 512:1024],
                                         in_=pr1[:], func=AF.Sigmoid)

            with tc.tile_pool(name="pxr", bufs=3) as pxr, \
                 tc.tile_pool(name="psr", bufs=2, space="PSUM") as psr:
                xr_saved = {}
                # ---------------- phase A: token-shift mix in d-major
                with (
                    tc.tile_pool(name="pxb", bufs=2) as pxb,
                    tc.tile_pool(name="pdx", bufs=1) as pdx,
                    tc.tile_pool(name="pxk", bufs=1) as pxk,
                    tc.tile_pool(name="pxtm", bufs=4) as pxtm,
                    tc.tile_pool(name="pst", bufs=4, space="PSUM") as pst,
                ):
                    xbufs = []
                    for ck in range(2):
                        xb = pxb.tile([P, DC, 1025], BF16, tag="xb")
                        nc.sync.dma_start(
                            out=xb[:],
                            in_=x_t[:, :, ck * 1024:ck * 1024 + 1025])
                        xbufs.append(xb)
                        if ck == 0:
                            nc.sync.dma_start(
                                out=wrt_sb[:],
                                in_=wrt.rearrange("(c p) e -> p c e", p=P))
                    for eli in range(EPC):
                        wk_sb[eli] = pwk.tile([P, DC, F], BF16, tag="wk",
                                              name=f"wk_sb{eli}")
                        wv_sb[eli] = pwv.tile([P, FC, D], BF16, tag="wv",
                                              name=f"wv_sb{eli}")
                        if eli == 0:
                            nc.sync.dma_start(
                                out=wk_sb[0][:],
                                in_=wk[0].rearrange("(c p) f -> p c f", p=P))
                            nc.sync.dma_start(
                                out=wv_sb[0][:],
                                in_=wv[0].rearrange("(c p) f -> p c f", p=P))

                    for ck in range(2):
                        xb = xbufs[ck]
                        for h in range(2):
                            hidx = ck * 2 + h
                            o = h * 512
                            dx = pdx.tile([P, DC, 512], BF16, tag="dx")
                            # dxprev = xprev - x
                            nc.vector.tensor_sub(
                                out=dx[:], in0=xb[:, :, o:o + 512],
                                in1=xb[:, :, o + 1:o + 513])
                            xk_h = pxk.tile([P, DC, 512], BF16, tag="xk")
                            xr_h = pxr.tile([P, DC, 512], BF16, tag="xr")
                            for c in range(DC):
                                nc.vector.scalar_tensor_tensor(
                                    out=xk_h[:, c, :], in0=dx[:, c, :],
                                    scalar=m2[:, c:c + 1],
                                    in1=xb[:, c, o + 1:o + 513],
                                    op0=ALU.mult, op1=ALU.add)
                                nc.vector.scalar_tensor_tensor(
                                    out=xr_h[:, c, :], in0=dx[:, c, :],
                                    scalar=m2[:, DC + c:DC + c + 1],
                                    in1=xb[:, c, o + 1:o + 513],
                                    op0=ALU.mult, op1=ALU.add)
                            xr_saved[hidx] = xr_h
                            # transpose xk to token-major, scatter rows
                            # into the (half, parity) dispatch buffer
                            with tc.high_priority():
                                for tt in range(4):
                                    ps = pst.tile([P, DC, P], BF16,
                                                  space="PSUM", tag="ps")
                                    for c in range(DC):
                                        nc.tensor.transpose(
                                            out=ps[:, c, :],
                                            in_=xk_h[:, c,
                                                     tt * P:(tt + 1) * P],
                                            identity=ident[:])
                                    xtm = pxtm.tile([P, D], BF16, tag="xtm")
                                    nc.vector.tensor_copy(out=xtm[:],
                                                          in_=ps[:])
                                    t = hidx * 4 + tt
                                    g = ck * 2 + t % 2
                                    nc.gpsimd.indirect_dma_start(
                                        out=a1[g][:],
                                        out_offset=bass.IndirectOffsetOnAxis(
                                            ap=s32[:, t:t + 1], axis=0),
                                        in_=xtm[:], in_offset=None)
                        for g in (ck * 2, ck * 2 + 1):
                            nc.gpsimd.collective_compute(
                                "AllToAll", mybir.AluOpType.bypass,
                                replica_groups=rg,
                                ins=[a1[g][0:NCORES * dK[g], :]],
                                outs=[recv1[dbase[g]:dbase[g + 1], :]])
                        if ck == 0:
                            # receptance fills the PE while the first
                            # dispatch collectives are in flight
                            recept(0, xr_saved[0], psr)
                            recept(1, xr_saved[1], psr)

                # deferred receptance hides the second dispatch pair
                recept(2, xr_saved[2], psr)
                recept(3, xr_saved[3], psr)

            # ---------------- phase C: expert FFNs
            with (
                tc.tile_pool(name="pfx", bufs=1) as pfx,
                tc.tile_pool(name="pfh", bufs=1) as pfh,
                tc.tile_pool(name="phr", bufs=2) as phr,
                tc.tile_pool(name="pfy", bufs=2) as pfy,
                tc.tile_pool(name="psh", bufs=2, space="PSUM") as psh,
                tc.tile_pool(name="psy", bufs=2, space="PSUM") as psy,
                tc.tile_pool(name="pd", bufs=3) as pd,
                tc.tile_pool(name="pdo", bufs=3) as pdo,
            ):
                def phase_d():
                    """gather own tokens' y rows, multiply by r"""
                    for ck in range(T // 512):
                        yg = pd.tile([P, 4, D], BF16, tag="yg")
                        nc.gpsimd.dma_gather(
                            out_ap=yg[:], in_ap=recv2[:],
                            idxs_ap=yg16[:, ck * 32:(ck + 1) * 32],
                            num_idxs=512, num_idxs_reg=512, elem_size=D,
                            transpose=False)
                        yo = pdo.tile([P, 4, D], BF16, tag="yo")
                        nc.vector.tensor_mul(
                            out=yo[:], in0=yg[:],
                            in1=r_sb[:, ck * 4:(ck + 1) * 4, :])
                        # SWDGE store casts bf16 -> fp32 on the way out
                        nc.gpsimd.dma_start(
                            out=out[ck * 512:(ck + 1) * 512, :].rearrange(
                                "(a p) d -> p a d", p=P),
                            in_=yo[:])

                # chunk list: (slot start, width, a-only). The first
                # chunk reads only the first-half region of recv1, so
                # its gather waits just the first two dispatch A2As.
                CH = [(0, 384, True), (384, 512, False), (896, 128, False)]
                ABASE = dbase[2]
                for el in range(EPC):
                    if el == 1:
                        nc.sync.dma_start(
                            out=wk_sb[1][:],
                            in_=wk[1].rearrange("(c p) f -> p c f", p=P))
                        nc.sync.dma_start(
                            out=wv_sb[1][:],
                            in_=wv[1].rearrange("(c p) f -> p c f", p=P))
                    XTs = []
                    for start, width, a_only in CH:
                        XT = pfx.tile([P, DC, width], BF16, tag=f"XT{width}",
                                      name=f"XT_{el}_{start}")
                        col0 = (el * C + start) // 16
                        src_ap = recv1[0:ABASE, :] if a_only else recv1[:]
                        nc.gpsimd.dma_gather(
                            out_ap=XT[:], in_ap=src_ap,
                            idxs_ap=sl16[:, col0:col0 + width // 16],
                            num_idxs=width, num_idxs_reg=width, elem_size=D,
                            transpose=True)
                        XTs.append(XT)
                    for ci, (start, width, a_only) in enumerate(CH):
                        XT = XTs[ci]
                        ht = pfh.tile([P, FC, 512], BF16, tag="ht")
                        for ft in range(FC):
                            ph = psh.tile([P, 512], F32, space="PSUM", tag="ph")
                            for dc in range(DC):
                                nc.tensor.matmul(
                                    out=ph[:, 0:width],
                                    lhsT=wk_sb[el][:, dc, ft * P:(ft + 1) * P],
                                    rhs=XT[:, dc, :],
                                    start=(dc == 0), stop=(dc == DC - 1))
                            hr = phr.tile([P, 512], BF16, tag="hr")
                            nc.scalar.activation(out=hr[:, 0:width],
                                                 in_=ph[:, 0:width],
                                                 func=AF.Relu)
                            nc.vector.tensor_mul(out=ht[:, ft, 0:width],
                                                 in0=hr[:, 0:width],
                                                 in1=hr[:, 0:width])
                        for tt in range(width // P):
                            ysb = pfy.tile([P, D], BF16, tag="ysb")
                            py0 = psy.tile([P, 512], F32, space="PSUM", tag="py0")
                            py1 = psy.tile([P, 512], F32, space="PSUM", tag="py1")
                            for fc in range(FC):
                                nc.tensor.matmul(
                                    out=py0[:],
                                    lhsT=ht[:, fc, tt * P:(tt + 1) * P],
                                    rhs=wv_sb[el][:, fc, 0:512],
                                    start=(fc == 0), stop=(fc == FC - 1))
                                nc.tensor.matmul(
                                    out=py1[:],
                                    lhsT=ht[:, fc, tt * P:(tt + 1) * P],
                                    rhs=wv_sb[el][:, fc, 512:1024],
                                    start=(fc == 0), stop=(fc == FC - 1))
                            nc.vector.tensor_copy(out=ysb[:, 0:512], in_=py0[:])
                            nc.vector.tensor_copy(out=ysb[:, 512:1024],
                                                  in_=py1[:])
                            scol = el * (C // P) + start // P + tt
                            nc.gpsimd.indirect_dma_start(
                                out=a2[el][:],
                                out_offset=bass.IndirectOffsetOnAxis(
                                    ap=s232[:, scol:scol + 1], axis=0),
                                in_=ysb[:], in_offset=None)
                    nc.gpsimd.collective_compute(
                        "AllToAll", mybir.AluOpType.bypass, replica_groups=rg,
                        ins=[a2[el][0:NCORES * K2[el], :]],
                        outs=[recv2[off2[el]:off2[el] + NCORES * K2[el], :]])

                # ---------------- phase D: gather own rows, multiply by r
                phase_d()

    nc.finalize()
    return nc


def _prepare_inputs(x, token_ids, shift_state, time_maa_k, time_maa_r,
                    w_recept, w_key, w_value):
    cfg, idxs = _build_indices(token_ids)
    x = np.asarray(x, np.float32)
    shift = np.asarray(shift_state, np.float32)
    wrt = np.ascontiguousarray(np.asarray(w_recept, np.float32).T).astype(nbf16)
    wkb = np.asarray(w_key, np.float32).astype(nbf16)
    wvb = np.asarray(w_value, np.float32).astype(nbf16)
    mk = np.asarray(time_maa_k, np.float32)
    mr = np.asarray(time_maa_r, np.float32)
    # [P, 2*DC]: col c = maa_k[c*128+p], col DC+c = maa_r[c*128+p]
    maa2 = np.concatenate(
        [mk.reshape(DC, P).T, mr.reshape(DC, P).T], axis=1).astype(nbf16)
    maa2 = np.ascontiguousarray(maa2)

    in_maps = []
    for k in range(NCORES):
        # x transposed, d-major: x_t[p, c, 1+t] = x[k, t, c*128+p]
        xk_full = np.concatenate([shift[k][:, None], x[k].T], axis=1)
        x_t = np.ascontiguousarray(
            xk_full.reshape(DC, P, T + 1).transpose(1, 0, 2)).astype(nbf16)
        in_maps.append({
            "x_t": x_t,
            "maa2": maa2, "wrt": wrt,
            "wk": np.ascontiguousarray(wkb[EPC * k:EPC * (k + 1)]),
            "wv": np.ascontiguousarray(wvb[EPC * k:EPC * (k + 1)]),
            **idxs[k],
        })
    return cfg, in_maps


def kernel(x, token_ids, shift_state, time_maa_k, time_maa_r,
           w_recept, w_key, w_value, _trace=False, _trace_cores=None):
    cfg, in_maps = _prepare_inputs(x, token_ids, shift_state, time_maa_k,
                                   time_maa_r, w_recept, w_key, w_value)
    if cfg not in _CACHE:
        _CACHE[cfg] = _build_nc(cfg)
    nc = _CACHE[cfg]
    res = run_bass_kernel_spmd(nc, in_maps, core_ids=list(range(NCORES)),
                               trace=_trace, trace_cores=_trace_cores)
    kernel.last_result = res
    y = np.stack([res.results[k]["out"] for k in range(NCORES)], axis=0)
    return y.astype(np.float32)
